# revision 32
# baseline (speedup 1.0000x reference)
"""Trainium2 Bass kernel for nn_InterpreMol_55877524521515.

6-layer post-norm transformer encoder, B=64 molecules, S=255(+CLS)=256,
D=512, H=8 heads, FF=2048, plus a 2-layer head on the CLS token.

v3 design:
- Data-parallel over batch (8 molecules/core); molecules processed in PAIRS
  so big matmuls stream N=512 moving columns (halves PE sequencer pressure).
- bf16 on the whole GEMM path (weights + activations); f32 PSUM; LN stat
  matmuls in f32r/bf16. Weights single-buffered (except the pp param pack),
  prefetched at emission points that hide the WAR wait.
- Engines are in-order, so EMISSION ORDER IS THE SCHEDULE: each layer is
  emitted as a software-pipelined wave plan that interleaves one pair's
  attention (act/vector-heavy) with another pair's projection/FFN matmuls
  (PE-heavy) to avoid head-of-line blocking on the PE queue.
- Engine placement constraints: GpSimd(Pool)=SBUF-only, DVE=the PSUM-reading
  elementwise ops, Act=exp/gelu/copies/per-channel scale-bias (table churn
  is just exp<->gelu).
- LN rstd = bit-hack Newton rsqrt (seed DVE, 1 NR step on Pool) - no sqrt
  table load on Act.
- Softmax denominator via ones-augmented V operand; ones columns and the
  rc2 zero lanes are initialized ONCE per pool buffer and never rewritten.
"""
import sys

sys.path.insert(0, "/opt/trn_rl_repo")

import numpy as np
import ml_dtypes

import concourse.bass as bass
import concourse.tile as tile
from concourse import bacc, mybir
from concourse.bass_utils import run_bass_kernel_spmd

F32 = mybir.dt.float32
F32R = mybir.dt.float32r
BF16 = mybir.dt.bfloat16
I32 = mybir.dt.int32
FP8 = mybir.dt.float8e4
PM = mybir.MatmulPerfMode
AF = mybir.ActivationFunctionType
OP = mybir.AluOpType
BFNP = ml_dtypes.bfloat16
F8NP = ml_dtypes.float8_e4m3

B, S, D, H, L, FF, HID = 64, 255, 512, 8, 6, 2048, 256
S1 = 256          # seq with CLS
BL = 8            # molecules per core
NP = 4            # molecule pairs per core
DK = D // H       # 64
NCORE = 8
EPS = 1e-5
NEG = -30000.0    # masked-key bias
MAGIC1 = 0x5F3759E0  # rsqrt magic + 1 (for the ~x + (MAGIC+1) form)

# pp param-pack column map (per layer, [128, 48]):
#  0:4 bq*0.125 | 4:8 bk | 8:12 bo | 12:28 b1 | 28:32 b2
#  32:36 ln1_g | 36:40 ln1_b | 40:44 ln2_g | 44:48 ln2_b
PPC = 48

POOL_CFG = [
    ("cst", dict(bufs=1)),
    ("qtp", dict(bufs=16)), ("ktp", dict(bufs=16)), ("vgp", dict(bufs=16)),
    ("btp", dict(bufs=3)), ("scp", dict(bufs=3)), ("exp_", dict(bufs=3)),
    ("atp", dict(bufs=8)), ("xap", dict(bufs=8)), ("xlp", dict(bufs=18)),
    ("htp", dict(bufs=17)), ("sqp", dict(bufs=4)), ("lnp", dict(bufs=7)),
    ("bcp", dict(bufs=3)), ("rcp", dict(bufs=1)),
    # lnp 8: two pairs' LN chains must rotate without cross-serializing
    ("psb", dict(bufs=3, space="PSUM")),
    ("pss", dict(bufs=2, space="PSUM")),
    ("psa", dict(bufs=2, space="PSUM")),
]


def build_program(reps=1):
    nc = bacc.Bacc("TRN2", target_bir_lowering=False, debug=False)

    x0t_d = nc.dram_tensor("x0t", [D, NP * 512], BF16, kind="ExternalInput")
    bias_d = nc.dram_tensor("biast", [BL, H, 128, 2, S1], BF16,
                            kind="ExternalInput")
    wq_d = nc.dram_tensor("wq", [L * D, D], BF16, kind="ExternalInput")
    wk_d = nc.dram_tensor("wk", [L * D, D], BF16, kind="ExternalInput")
    wv_d = nc.dram_tensor("wv", [L * D, D], BF16, kind="ExternalInput")
    wo_d = nc.dram_tensor("wo", [L * D, D], BF16, kind="ExternalInput")
    w1_d = nc.dram_tensor("w1", [L * D, FF], BF16, kind="ExternalInput")
    w2_d = nc.dram_tensor("w2", [L * FF, D], BF16, kind="ExternalInput")
    pp_d = nc.dram_tensor("pp", [L * 128, PPC], F32, kind="ExternalInput")
    hw1_d = nc.dram_tensor("hw1", [D, HID], BF16, kind="ExternalInput")
    hb1_d = nc.dram_tensor("hb1", [128, 2], F32, kind="ExternalInput")
    hw2_d = nc.dram_tensor("hw2", [128, 2], BF16, kind="ExternalInput")
    hb2_d = nc.dram_tensor("hb2", [1, 1], F32, kind="ExternalInput")
    out_d = nc.dram_tensor("out", [1, BL], F32, kind="ExternalOutput")

    from contextlib import ExitStack
    with tile.TileContext(nc) as tc:
        with ExitStack() as stack:
            P = {}
            for pname, kw in POOL_CFG:
                P[pname] = stack.enter_context(tc.tile_pool(name=pname, **kw))
            cst, qtp, ktp, vgp = P["cst"], P["qtp"], P["ktp"], P["vgp"]
            btp, scp, exp_, atp = P["btp"], P["scp"], P["exp_"], P["atp"]
            xap, xlp, htp, sqp = P["xap"], P["xlp"], P["htp"], P["sqp"]
            lnp, bcp, rcp = P["lnp"], P["bcp"], P["rcp"]
            psb, pss, psa = P["psb"], P["pss"], P["psa"]

            # ---- static tiles -------------------------------------------
            xres = [[cst.tile([128, 512], BF16, name=f"xres_{kt}_{p}")
                     for p in range(NP)] for kt in range(4)]
            wq_sb = [cst.tile([128, D], BF16, name=f"wq_sb{kt}")
                     for kt in range(4)]
            wk_sb = [cst.tile([128, D], BF16, name=f"wk_sb{kt}")
                     for kt in range(4)]
            wv_sb = [cst.tile([128, D], BF16, name=f"wv_sb{kt}")
                     for kt in range(4)]
            wo_sb = [cst.tile([128, D], BF16, name=f"wo_sb{kt}")
                     for kt in range(4)]
            pp_sb = [cst.tile([128, PPC], F32, name=f"pp_sb{pr}")
                     for pr in range(2)]
            w1_sb = [cst.tile([128, FF], BF16, name=f"w1_sb{kt}")
                     for kt in range(4)]
            w2_sb = [cst.tile([128, D], BF16, name=f"w2_sb{kt}")
                     for kt in range(16)]
            ones_inv_r = cst.tile([128, 128], F32R, name="ones_inv_r")
            ones_inv_b = cst.tile([128, 128], BF16, name="ones_inv_b")
            ones2 = cst.tile([1, 64], F32R, name="ones2")
            ones8 = cst.tile([128, 8], BF16, name="ones8")
            hw1_sb = [cst.tile([128, HID], BF16, name=f"hw1_sb{kt}")
                      for kt in range(4)]
            hb1_sb = cst.tile([128, 2], F32, name="hb1_sb")
            hw2_sb = cst.tile([128, 2], BF16, name="hw2_sb")
            hb2_sb = cst.tile([1, 1], F32, name="hb2_sb")
            cls_sb = [cst.tile([128, BL], BF16, name=f"cls_sb{kt}")
                      for kt in range(4)]
            h_sb = [cst.tile([128, BL], BF16, name=f"h_sb{mt}")
                    for mt in range(2)]
            out_sb = cst.tile([1, BL], F32, name="out_sb")

            # ---- constants ----------------------------------------------
            cinit = cst.tile([128, 128], F32, name="cinit")
            nc.vector.memset(cinit[:], 1.0 / D)
            nc.vector.tensor_copy(ones_inv_r[:], cinit[:])
            nc.vector.tensor_copy(ones_inv_b[:], cinit[:])
            nc.vector.memset(cinit[:], 1.0)
            nc.vector.tensor_copy(ones2[:], cinit[0:1, 0:64])
            nc.vector.tensor_copy(ones8[:], cinit[:, 0:8])

            # Pre-touch rotating pool buffers whose constant regions are
            # written once and then only read: vg ones column (softmax
            # denominator operand) and the bc zero lanes (rc2 off-diagonal).
            vg_bufs = dict(POOL_CFG)["vgp"]["bufs"]
            for _ in range(vg_bufs):
                vginit = vgp.tile([128, H, DK + 1], BF16, name="vg")
                nc.vector.tensor_copy(
                    vginit[:, :, DK:DK + 1],
                    ones8[:].rearrange("p (h o) -> p h o", o=1))

            # ---- initial loads ------------------------------------------
            for kt in range(4):
                for p in range(NP):
                    nc.sync.dma_start(
                        out=xres[kt][p][:],
                        in_=x0t_d.ap()[kt * 128:(kt + 1) * 128,
                                       p * 512:(p + 1) * 512])
            for kt in range(4):
                nc.sync.dma_start(out=hw1_sb[kt][:],
                                  in_=hw1_d.ap()[kt * 128:(kt + 1) * 128, :])
            nc.sync.dma_start(out=hb1_sb[:], in_=hb1_d.ap())
            nc.sync.dma_start(out=hw2_sb[:], in_=hw2_d.ap())
            nc.sync.dma_start(out=hb2_sb[:], in_=hb2_d.ap())

            def load_qkv(l):
                for dst, src in ((wq_sb, wq_d), (wk_sb, wk_d), (wv_sb, wv_d)):
                    for kt in range(4):
                        nc.sync.dma_start(
                            out=dst[kt][:],
                            in_=src.ap()[l * D + kt * 128:
                                         l * D + (kt + 1) * 128, :])

            def load_wo(l):
                for kt in range(4):
                    nc.sync.dma_start(out=wo_sb[kt][:],
                                      in_=wo_d.ap()[l * D + kt * 128:
                                                    l * D + (kt + 1) * 128,
                                                    :])

            def load_w1(l):
                for kt in range(4):
                    nc.sync.dma_start(out=w1_sb[kt][:],
                                      in_=w1_d.ap()[l * D + kt * 128:
                                                    l * D + (kt + 1) * 128,
                                                    :])

            def load_w2(l):
                for kt in range(16):
                    nc.sync.dma_start(out=w2_sb[kt][:],
                                      in_=w2_d.ap()[l * FF + kt * 128:
                                                    l * FF + (kt + 1) * 128,
                                                    :])

            def load_pp(l):
                pr = l % 2
                nc.sync.dma_start(out=pp_sb[pr][:],
                                  in_=pp_d.ap()[l * 128:(l + 1) * 128, :])

            # ---- per-pair state + emitter units -------------------------
            st8 = {}

            def qkv_units(pr, p):
                """12 emitter closures: 4 q, 4 k, 4 v(half,st)."""
                st8[p] = dict(q=[None] * 4, k=[None] * 4,
                              vg=[[None, None], [None, None]])
                units = []

                def q_unit(mt):
                    ps_q = psb.tile([128, 512], F32, name="ps_q", tag="big")
                    for kt in range(4):
                        nc.tensor.matmul(
                            ps_q[:], wq_sb[kt][:, mt * 128:(mt + 1) * 128],
                            xres[kt][p][:], start=(kt == 0), stop=(kt == 3))
                    q = qtp.tile([128, 512], BF16, name="q")
                    with nc.allow_low_precision(reason="bf16 act"):
                        nc.vector.tensor_scalar(
                            q[:], ps_q[:], pp_sb[pr][:, mt:mt + 1], None,
                            op0=OP.add)
                    st8[p]["q"][mt] = q

                def k_unit(mt):
                    ps_k = psb.tile([128, 512], F32, name="ps_k", tag="big")
                    for kt in range(4):
                        nc.tensor.matmul(
                            ps_k[:], wk_sb[kt][:, mt * 128:(mt + 1) * 128],
                            xres[kt][p][:], start=(kt == 0), stop=(kt == 3))
                    k = ktp.tile([128, 512], BF16, name="k")
                    with nc.allow_low_precision(reason="bf16 act"):
                        nc.vector.tensor_scalar(
                            k[:], ps_k[:], pp_sb[pr][:, 4 + mt:5 + mt], None,
                            op0=OP.add)
                    st8[p]["k"][mt] = k

                def v_unit(half, st):
                    ps_v = psb.tile([128, 512], F32, name="ps_v", tag="big")
                    c0 = half * 256 + st * 128
                    for kt in range(4):
                        nc.tensor.matmul(
                            ps_v[:], xres[kt][p][:, c0:c0 + 128],
                            wv_sb[kt][:], start=(kt == 0), stop=(kt == 3))
                    vgt = vgp.tile([128, H, DK + 1], BF16, name="vg")
                    # bv is folded into bo on the host (softmax weights
                    # sum to 1), so no ones-row bias matmul is needed
                    with nc.allow_low_precision(reason="bf16 act"):
                        nc.scalar.activation(
                            vgt[:, :, 0:DK],
                            ps_v[:].rearrange("p (h d) -> p h d", h=H),
                            AF.Identity)
                    st8[p]["vg"][half][st] = vgt

                for mt in range(4):
                    units.append(lambda mt=mt: q_unit(mt))
                    units.append(lambda mt=mt: k_unit(mt))
                for half in range(2):
                    for st in range(2):
                        units.append(
                            lambda half=half, st=st: v_unit(half, st))
                return units

            def attn_start(p):
                st8[p]["at"] = [atp.tile([128, 512], BF16, name="at")
                                for _ in range(4)]

            def attn_unit(p, h):
                s = st8[p]
                q_t, k_t, vg, at_t = s["q"], s["k"], s["vg"], s["at"]
                r0 = (h % 2) * 64
                mth = h // 2
                ps_av = psa.tile([128, 2, S1], F32, name="ps_av", tag="av")
                # emit both halves' scores+softmax first so half-b's score
                # matmuls cover half-a's add->exp latency on the PE queue
                exs = []
                for half in range(2):
                    bt = btp.tile([128, 2, S1], BF16, name="bt")
                    nc.sync.dma_start(out=bt[:],
                                      in_=bias_d.ap()[p * 2 + half, h])
                    ps_sc = pss.tile([128, 2, S1], F32, name="ps_sc",
                                     tag="sc")
                    for st in range(2):
                        c0 = half * 256 + st * 128
                        nc.tensor.matmul(
                            ps_sc[:, st, :],
                            k_t[mth][r0:r0 + 64, c0:c0 + 128],
                            q_t[mth][r0:r0 + 64,
                                     half * 256:(half + 1) * 256],
                            start=True, stop=True)
                    sc = scp.tile([128, 2, S1], BF16, name="sc")
                    with nc.allow_low_precision(reason="bf16 logits"):
                        nc.vector.tensor_add(sc[:], ps_sc[:], bt[:])
                    ex = exp_.tile([128, 2, S1], BF16, name="ex")
                    with nc.allow_low_precision(reason="bf16 softmax"):
                        nc.scalar.activation(ex[:], sc[:], AF.Exp)
                    exs.append(ex)
                for half in range(2):
                    for st in range(2):
                        nc.tensor.matmul(
                            ps_av[0:DK + 1, half, :],
                            vg[half][st][:, h, :],
                            exs[half][:, st, :],
                            start=(st == 0), stop=(st == 1))
                rcr = rcp.tile([1, 2, S1], F32R, name="rc")
                with nc.allow_low_precision(reason="softmax recip"):
                    nc.vector.reciprocal(rcr[:, 0, :],
                                         ps_av[DK:DK + 1, 0, :])
                    nc.vector.reciprocal(rcr[:, 1, :],
                                         ps_av[DK:DK + 1, 1, :])
                ps_bc = psa.tile([64, 2, S1], F32, name="ps_bc", tag="bcps",
                                 bufs=1)
                nc.tensor.matmul(ps_bc[:], ones2[:], rcr[:],
                                 start=True, stop=True)
                bc = bcp.tile([64, 2, S1], BF16, name="bc")
                with nc.allow_low_precision(reason="bf16 bc"):
                    nc.scalar.activation(bc[:], ps_bc[:], AF.Identity)
                with nc.allow_low_precision(reason="bf16 attn"):
                    nc.vector.tensor_mul(
                        at_t[mth][r0:r0 + 64, :].rearrange(
                            "p (a q) -> p a q", a=2),
                        ps_av[0:DK, :, :],
                        bc[:, :, :])

            def o_proj(pr, p):
                s = st8[p]
                xa_t = []
                for mt in range(4):
                    ps_o = psb.tile([128, 512], F32, name="ps_o", tag="big")
                    for kt in range(4):
                        nc.tensor.matmul(
                            ps_o[:], wo_sb[kt][:, mt * 128:(mt + 1) * 128],
                            s["at"][kt][:], start=(kt == 0), stop=(kt == 3))
                    xa = xap.tile([128, 512], F32R, name="xa", tag="xa")
                    nc.vector.scalar_tensor_tensor(
                        xa[:], ps_o[:], pp_sb[pr][:, 8 + mt:9 + mt],
                        xres[mt][p][:], op0=OP.add, op1=OP.add)
                    xa_t.append(xa)
                s["xa"] = xa_t

            def layer_norm(a_t, pr, gcol, dst_tiles):
                ps_m = psb.tile([128, 512], F32, name="ps_m", tag="big")
                for kt in range(4):
                    nc.tensor.matmul(ps_m[:], ones_inv_r[:], a_t[kt][:],
                                     start=(kt == 0), stop=(kt == 3))
                sq_t = []
                for kt in range(4):
                    sq = sqp.tile([128, 512], BF16, name="sq")
                    with nc.allow_low_precision(reason="bf16 sq"):
                        nc.gpsimd.tensor_tensor(sq[:], a_t[kt][:],
                                                a_t[kt][:], op=OP.mult)
                    sq_t.append(sq)
                ps_s = psb.tile([128, 512], F32, name="ps_s", tag="big")
                for kt in range(4):
                    nc.tensor.matmul(ps_s[:], ones_inv_b[:], sq_t[kt][:],
                                     start=(kt == 0), stop=(kt == 3))
                mean_sb = lnp.tile([128, 512], F32, name="mean_sb", tag="ln")
                nc.scalar.activation(mean_sb[:], ps_m[:], AF.Identity)
                m2 = lnp.tile([128, 512], F32, name="m2", tag="ln")
                nc.scalar.activation(m2[:], ps_m[:], AF.Square)
                var = lnp.tile([128, 512], F32, name="var", tag="ln")
                nc.vector.scalar_tensor_tensor(var[:], ps_s[:], EPS, m2[:],
                                               op0=OP.add, op1=OP.subtract)
                y0 = lnp.tile([128, 512], F32, name="y0", tag="ln")
                nc.vector.tensor_scalar(
                    y0[:].bitcast(I32), var[:].bitcast(I32), 1, -1,
                    op0=OP.arith_shift_right, op1=OP.bitwise_xor)
                y = lnp.tile([128, 512], F32, name="y1", tag="ln")
                nc.gpsimd.tensor_scalar(
                    y[:].bitcast(I32), y0[:].bitcast(I32), MAGIC1, None,
                    op0=OP.add)
                # one Newton step: y = y*(1.5 - 0.5*var*y*y)
                t = lnp.tile([128, 512], F32, name="t", tag="ln")
                nc.gpsimd.tensor_tensor(t[:], y[:], y[:], op=OP.mult)
                nc.gpsimd.tensor_tensor(t[:], t[:], var[:], op=OP.mult)
                w = lnp.tile([128, 512], F32, name="w", tag="ln")
                nc.gpsimd.tensor_scalar(w[:], t[:], -0.5, 1.5,
                                        op0=OP.mult, op1=OP.add)
                yn = lnp.tile([128, 512], F32, name="yn", tag="ln")
                nc.gpsimd.tensor_tensor(yn[:], y[:], w[:], op=OP.mult)
                outs = []
                for kt in range(4):
                    cen = lnp.tile([128, 512], F32, name="cen", tag="ln")
                    nc.gpsimd.tensor_sub(cen[:], a_t[kt][:], mean_sb[:])
                    nrm = lnp.tile([128, 512], F32, name="nrm", tag="ln")
                    nc.gpsimd.tensor_mul(nrm[:], cen[:], yn[:])
                    if dst_tiles is None:
                        o = xlp.tile([128, 512], BF16, name="xl")
                        outs.append(o)
                        dst = o[:]
                    else:
                        dst = dst_tiles[kt][:]
                    with nc.allow_low_precision(reason="bf16 ln"):
                        nc.scalar.activation(
                            dst, nrm[:], AF.Identity,
                            bias=pp_sb[pr][:, gcol + 4 + kt:gcol + 5 + kt],
                            scale=pp_sb[pr][:, gcol + kt:gcol + 1 + kt])
                return outs

            def ln1(pr, p):
                st8[p]["xl"] = layer_norm(st8[p]["xa"], pr, 32, None)

            def ffn1_start(p):
                st8[p]["ht"] = [None] * 16

            def ffn1_unit(pr, p, fb):
                s = st8[p]
                ps_f = psb.tile([128, 512], F32, name="ps_f", tag="big")
                for kt in range(4):
                    nc.tensor.matmul(
                        ps_f[:], w1_sb[kt][:, fb * 128:(fb + 1) * 128],
                        s["xl"][kt][:], start=(kt == 0), stop=(kt == 3))
                ht = htp.tile([128, 512], BF16, name="ht")
                with nc.allow_low_precision(reason="bf16 ffn"):
                    nc.scalar.activation(ht[:], ps_f[:], AF.Gelu,
                                         bias=pp_sb[pr][:, 12 + fb:13 + fb])
                s["ht"][fb] = ht

            def ffn2(pr, p):
                s = st8[p]
                xb_t = []
                for mt in range(4):
                    ps_g = psb.tile([128, 512], F32, name="ps_g", tag="big")
                    for kt in range(16):
                        nc.tensor.matmul(
                            ps_g[:], w2_sb[kt][:, mt * 128:(mt + 1) * 128],
                            s["ht"][kt][:], start=(kt == 0), stop=(kt == 15))
                    xb = xap.tile([128, 512], F32R, name="xb", tag="xa")
                    nc.vector.scalar_tensor_tensor(
                        xb[:], ps_g[:], pp_sb[pr][:, 28 + mt:29 + mt],
                        s["xl"][mt][:], op0=OP.add, op1=OP.add)
                    xb_t.append(xb)
                s["xb"] = xb_t

            def ln2(pr, p):
                layer_norm(st8[p]["xb"], pr, 40,
                           [xres[kt][p] for kt in range(4)])

            def interleave(main_units, fill_units):
                """Emit main units, distributing fill units evenly between."""
                nm, nf = len(main_units), len(fill_units)
                fi = 0
                for i, u in enumerate(main_units):
                    u()
                    want = (i + 1) * nf // nm
                    while fi < want:
                        fill_units[fi]()
                        fi += 1

            def layer_body(l, prefetch):
                pr = l % 2
                # W0: QKV for pairs 0,1
                for u in qkv_units(pr, 0):
                    u()
                for u in qkv_units(pr, 1):
                    u()
                # W1: attention(0,1) interleaved with QKV(2)
                attn_start(0)
                attn_start(1)
                mains = []
                for h in range(8):
                    mains.append(lambda h=h: attn_unit(0, h))
                    mains.append(lambda h=h: attn_unit(1, h))
                interleave(mains, qkv_units(pr, 2) + qkv_units(pr, 3))
                # W2: o/ln1(0,1); qkv weights now dead -> prefetch next
                # layer's q/k/v (hidden behind W3+)
                o_proj(pr, 0)
                ln1(pr, 0)
                o_proj(pr, 1)
                ln1(pr, 1)
                if prefetch:
                    load_qkv((l + 1) % L)
                    load_pp((l + 1) % L)
                # W3: attention(2,3) interleaved with FFN1(0)
                attn_start(2)
                attn_start(3)
                ffn1_start(0)
                mains = []
                for h in range(8):
                    mains.append(lambda h=h: attn_unit(2, h))
                    mains.append(lambda h=h: attn_unit(3, h))
                fills = [lambda fb=fb: ffn1_unit(pr, 0, fb)
                         for fb in range(16)]
                for blk in range(4):
                    for u in mains[blk * 4:(blk + 1) * 4]:
                        u()
                    for u in fills[blk * 4:(blk + 1) * 4]:
                        u()
                # W4: o/ln1(2,3); wo now dead -> prefetch
                o_proj(pr, 2)
                ln1(pr, 2)
                o_proj(pr, 3)
                ln1(pr, 3)
                if prefetch:
                    load_wo((l + 1) % L)
                # W5+: FFN2(p) then FFN1(p+1), pairwise, LN2 interleaved
                for p in range(NP):
                    ffn2(pr, p)
                    ln2(pr, p)
                    if p + 1 < NP:
                        ffn1_start(p + 1)
                        for fb in range(16):
                            ffn1_unit(pr, p + 1, fb)
                if prefetch:
                    load_w1((l + 1) % L)
                    load_w2((l + 1) % L)

            # ---- preamble weight loads ----------------------------------
            load_qkv(0)
            load_pp(0)
            load_wo(0)
            load_w1(0)
            load_w2(0)

            if reps > 1:
                with tc.For_i(0, reps, 1):
                    for l in range(L):
                        layer_body(l, True)
            else:
                for l in range(L):
                    layer_body(l, l < L - 1)

            # ---- head on CLS tokens -------------------------------------
            for kt in range(4):
                for p in range(NP):
                    for half in range(2):
                        m = p * 2 + half
                        nc.gpsimd.tensor_copy(
                            cls_sb[kt][:, m:m + 1],
                            xres[kt][p][:, half * 256:half * 256 + 1])
            for mt in range(2):
                ps_h = psb.tile([128, 512], F32, name="ps_h", tag="big")
                for kt in range(4):
                    nc.tensor.matmul(
                        ps_h[:, 0:BL],
                        hw1_sb[kt][:, mt * 128:(mt + 1) * 128],
                        cls_sb[kt][:],
                        start=(kt == 0), stop=(kt == 3))
                with nc.allow_low_precision(reason="bf16 head"):
                    nc.scalar.activation(h_sb[mt][:], ps_h[:, 0:BL], AF.Gelu,
                                         bias=hb1_sb[:, mt:mt + 1])
            ps_out = psb.tile([128, 512], F32, name="ps_out", tag="big")
            for mt in range(2):
                nc.tensor.matmul(ps_out[0:1, 0:BL], hw2_sb[:, mt:mt + 1],
                                 h_sb[mt][:], start=(mt == 0), stop=(mt == 1))
            nc.scalar.activation(out_sb[:], ps_out[0:1, 0:BL], AF.Identity,
                                 bias=hb2_sb[0:1, 0:1])
            nc.sync.dma_start(out=out_d.ap(), in_=out_sb[:])

    nc.compile()
    return nc


_CACHE = {}


def _get_program(reps):
    if reps not in _CACHE:
        _CACHE[reps] = build_program(reps)
    return _CACHE[reps]


def prep_inputs(atom_emb, edge_bias, key_padding_mask, cls_token, Wq, bq, Wk,
                bk, Wv, bv, Wo, bo, ln1_g, ln1_b, W1, b1, W2, b2, ln2_g,
                ln2_b, head_W1, head_b1, head_W2, head_b2):
    f32 = np.float32
    atom_emb = np.asarray(atom_emb, f32)
    cls_token = np.asarray(cls_token, f32)
    x0 = np.concatenate(
        [np.broadcast_to(cls_token, (B, 1, D)), atom_emb], axis=1)  # [B,S1,D]

    # biasT[b,h,k,q] = edge_bias[b,q-1,k-1,h]; masked key rows -> NEG;
    # reshaped to [B, H, 128, 2(st), 256] (k = st*128 + kp)
    bt = np.zeros((B, H, S1, S1), f32)
    eb = np.asarray(edge_bias, f32).transpose(0, 3, 2, 1)  # [b,h,j(k),i(q)]
    bt[:, :, 1:, 1:] = eb
    km = np.asarray(key_padding_mask, bool)
    bi, ki = np.nonzero(km)
    bt[bi, :, ki + 1, :] = NEG
    bt = bt.reshape(B, H, 2, 128, S1).transpose(0, 1, 3, 2, 4)
    bt = np.ascontiguousarray(bt).astype(BFNP)

    def seg(x):  # [L, dim] -> [L, 128, dim//128]
        x = np.asarray(x, f32)
        return x.reshape(L, -1, 128).transpose(0, 2, 1)

    pp = np.zeros((L, 128, PPC), f32)
    pp[:, :, 0:4] = seg(np.asarray(bq, f32) * 0.125)
    pp[:, :, 4:8] = seg(bk)
    bo_eff = np.asarray(bo, f32) + np.einsum(
        "ld,lde->le", np.asarray(bv, f32), np.asarray(Wo, f32))
    pp[:, :, 8:12] = seg(bo_eff)
    pp[:, :, 12:28] = seg(b1)
    pp[:, :, 28:32] = seg(b2)
    pp[:, :, 32:36] = seg(ln1_g)
    pp[:, :, 36:40] = seg(ln1_b)
    pp[:, :, 40:44] = seg(ln2_g)
    pp[:, :, 44:48] = seg(ln2_b)

    def bfm(x, shape):
        return np.ascontiguousarray(
            np.asarray(x, f32).reshape(shape)).astype(BFNP)

    shared = {
        "wq": bfm(np.asarray(Wq, f32) * 0.125, (L * D, D)),
        "wk": bfm(Wk, (L * D, D)),
        "wv": bfm(Wv, (L * D, D)),
        "wo": bfm(Wo, (L * D, D)),
        "w1": bfm(W1, (L * D, FF)),
        "w2": bfm(W2, (L * FF, D)),
        "pp": np.ascontiguousarray(pp.reshape(L * 128, PPC)),
        "hw1": bfm(head_W1, (D, HID)),
        "hb1": np.ascontiguousarray(
            np.asarray(head_b1, f32).reshape(2, 128).T),
        "hw2": bfm(np.asarray(head_W2, f32).reshape(2, 128).T, (128, 2)),
        "hb2": np.asarray(head_b2, f32).reshape(1, 1),
    }
    in_maps = []
    for c in range(NCORE):
        sl = slice(c * BL, (c + 1) * BL)
        x0t = np.ascontiguousarray(
            x0[sl].transpose(2, 0, 1).reshape(D, BL * S1)).astype(BFNP)
        in_maps.append({"x0t": x0t, "biast": np.ascontiguousarray(bt[sl]),
                        **shared})
    return in_maps


def run(in_maps, reps=1):
    nc = _get_program(reps)
    res = run_bass_kernel_spmd(nc, in_maps, list(range(NCORE)))
    out = np.concatenate([res.results[c]["out"].reshape(BL, 1)
                          for c in range(NCORE)], axis=0)
    return out


def kernel(**inputs) -> np.ndarray:
    in_maps = prep_inputs(**inputs)
    return run(in_maps, reps=1)
